# revision 38
# baseline (speedup 1.0000x reference)
"""MHA kernel for Trainium2, 8 NeuronCores.

Sharding: (batch, query-half) -> 8 shards. Core c handles batch c//2,
query rows (c%2)*1024:(c%2+1)*1024, all 16 heads. K/V projections are
SPLIT between the two cores sharing a batch (by kpos / s half) and
exchanged with pair-wise AllGather collectives (replica groups
[[0,1],[2,3],[4,5],[6,7]]), halving the K/V projection matmul work.
AllGather output is rank-ordered identically on both cores, giving both
the full K/V in global kpos order (attention is permutation-invariant
over kpos anyway). Output rows are disjoint across cores.

Host-side marshaling: X slices transposed and cast to bf16 ([D, S*]
layout, K/V halves per core); weights pre-packed into SBUF layouts.

Per-core compute (bf16 matmuls, f32 PSUM):
  K proj (own half, all 8 head-pairs) up-front -> AllGather in 2 chunks;
  V proj (own half) per quad -> AllGather; quad1's V proj emitted before
  head-pair 3's attention so the collective overlaps it.
  Per head-pair: q^T via woven half-psum-groups; scores^T per head via
  K=64 matmuls (head pair concurrent in PE row-tile groups);
  E = exp(scores^T) on ScalarE; z'^T accumulated with the ones-column
  trick; z^T = z'^T[:64] * recip(z'^T[64]) (approx_fast via SBUF staging,
  GpSimd partition-broadcast); out = z^T.T @ Wo + bo.
"""

import os

import numpy as np
import ml_dtypes



import concourse.bass as bass
import concourse.tile as tile
from concourse import bacc, mybir
from concourse.bass import ds, ts
from concourse.bass_utils import run_bass_kernel_spmd

B, S, D = 4, 2048, 1024
H, DK, DV = 16, 64, 64
N_CORES = 8
SQ = S // 2  # query rows per core
P = 128
NHP = H // 2  # head pairs
F32 = mybir.dt.float32
BF = mybir.dt.bfloat16
EXP = mybir.ActivationFunctionType.Exp
PAIRS = [[0, 1], [2, 3], [4, 5], [6, 7]]


def build_kernel(nc, tc, VARIANT=""):
    # pre-transposed bf16 inputs (K/V are the core's HALF: kpos/s slice)
    xqT_d = nc.declare_dram_parameter("xqT", [D, SQ], BF, isOutput=False).ap()
    xkT_d = nc.declare_dram_parameter("xkT", [D, SQ], BF, isOutput=False).ap()
    xvT_d = nc.declare_dram_parameter("xvT", [D, SQ], BF, isOutput=False).ap()
    # weights pre-packed [128, 8 dchunk, 1024 (h k)] bf16
    wq_d = nc.declare_dram_parameter("wq", [P, 8, H * DK], BF, isOutput=False).ap()
    wk_d = nc.declare_dram_parameter("wk", [P, 8, H * DK], BF, isOutput=False).ap()
    wv_d = nc.declare_dram_parameter("wv", [P, 8, H * DV], BF, isOutput=False).ap()
    # biases pre-packed [128 (pair-stacked), 8 hp] f32
    bq_d = nc.declare_dram_parameter("bq", [P, NHP], F32, isOutput=False).ap()
    bk_d = nc.declare_dram_parameter("bk", [P, NHP], F32, isOutput=False).ap()
    bvr_d = nc.declare_dram_parameter("bvr", [1, H * DV], BF, isOutput=False).ap()
    # Wo pre-packed [128, 8 fchunk, 1024 dout] bf16; bo [1, D] bf16
    wo_d = nc.declare_dram_parameter("wo", [P, 8, D], BF, isOutput=False).ap()
    bo_d = nc.declare_dram_parameter("bo", [1, D], BF, isOutput=False).ap()
    out = nc.declare_dram_parameter("out", [SQ, D], F32, isOutput=True).ap()

    import contextlib

    ctx = contextlib.ExitStack()
    with ctx:
        consts = ctx.enter_context(tc.tile_pool(name="consts", bufs=1))
        wpool = ctx.enter_context(tc.tile_pool(name="wpool", bufs=1))
        xtp = ctx.enter_context(tc.tile_pool(name="xtp", bufs=1))
        kp = ctx.enter_context(tc.tile_pool(name="kp", bufs=1))
        ztp = ctx.enter_context(tc.tile_pool(name="ztp", bufs=1))
        qkv = ctx.enter_context(tc.tile_pool(name="qkv", bufs=3))
        epool = ctx.enter_context(tc.tile_pool(name="epool", bufs=2))
        rpool = ctx.enter_context(tc.tile_pool(name="rpool", bufs=1))
        opool = ctx.enter_context(tc.tile_pool(name="opool", bufs=2))
        wopool = ctx.enter_context(tc.tile_pool(name="wopool", bufs=1))
        dram = ctx.enter_context(tc.tile_pool(name="dram", bufs=1, space="DRAM"))
        # PSUM: 8 banks = pp 1 + sp 4 + zp 3. zp=3 lets the next qt's AV
        # accumulation start while the previous qt's normalization chain
        # is still draining.
        pp = ctx.enter_context(tc.tile_pool(name="pp", bufs=1, space=bass.MemorySpace.PSUM))
        sp = ctx.enter_context(tc.tile_pool(name="sp", bufs=2, space=bass.MemorySpace.PSUM))
        zp = ctx.enter_context(tc.tile_pool(name="zp", bufs=3, space=bass.MemorySpace.PSUM))

        # ---- constants ----
        bqc = consts.tile([P, NHP], F32, tag="bqc")
        bkc = consts.tile([P, NHP], F32, tag="bkc")
        nc.gpsimd.dma_start(out=bqc[:, :], in_=bq_d[:, :])
        nc.gpsimd.dma_start(out=bkc[:, :], in_=bk_d[:, :])
        # staging rows ride the rpool "rb" ring slot (first real rb use is
        # far later; bufs=1 ring serializes the overwrites correctly)
        bvr = rpool.tile([1, D], BF, tag="rb")
        nc.gpsimd.dma_start(out=bvr[0:1, 0:H * DV], in_=bvr_d[0:1, :])
        bvb = consts.tile([P, H * DV], BF, tag="bvb")
        nc.gpsimd.partition_broadcast(bvb[:, :], bvr[0:1, 0:H * DV])
        bo_row = rpool.tile([1, D], BF, tag="rb")
        nc.gpsimd.dma_start(out=bo_row[0:1, :], in_=bo_d[0:1, :])
        bo_bc = consts.tile([P, D], BF, tag="bo_bc")
        nc.gpsimd.partition_broadcast(bo_bc[:, :], bo_row[0:1, :])

        wq_sb = wpool.tile([P, 8, H * DK], BF, tag="wq_sb")
        wk_sb = wpool.tile([P, 8, H * DK], BF, tag="wk_sb")
        wv_sb = wpool.tile([P, 8, H * DV], BF, tag="wv_sb")

        xqT = xtp.tile([P, 8, SQ], BF, tag="xqT")
        xkT = xtp.tile([P, 8, SQ], BF, tag="xkT")
        xvT = xtp.tile([P, 8, SQ], BF, tag="xvT")

        # load order = first-use order: q-proj starts the PE earliest, then
        # V (gates the longest-flight AllGather), then K
        nc.gpsimd.dma_start(out=wq_sb[:, 0:4, :], in_=wq_d[:, 0:4, :])
        for dc in range(4):
            eng = nc.sync if dc % 2 == 0 else nc.scalar
            eng.dma_start(out=xqT[:, dc, :], in_=xqT_d[ds(dc * P, P), :])
        nc.gpsimd.dma_start(out=wq_sb[:, 4:8, :], in_=wq_d[:, 4:8, :])
        for dc in range(4, 8):
            eng = nc.sync if dc % 2 == 0 else nc.scalar
            eng.dma_start(out=xqT[:, dc, :], in_=xqT_d[ds(dc * P, P), :])
        nc.gpsimd.dma_start(out=wv_sb[:, :, :], in_=wv_d[:, :, :])
        for dc in range(8):
            eng = nc.sync if dc % 2 == 0 else nc.scalar
            eng.dma_start(out=xvT[:, dc, :], in_=xvT_d[ds(dc * P, P), :])
        nc.gpsimd.dma_start(out=wk_sb[:, :, :], in_=wk_d[:, :, :])
        for dc in range(8):
            eng = nc.sync if dc % 2 == 0 else nc.scalar
            eng.dma_start(out=xkT[:, dc, :], in_=xkT_d[ds(dc * P, P), :])

        wo_sb0 = wopool.tile([P, 8, 512], BF, tag="wo_sb")
        wo_sb1 = wopool.tile([P, 8, 512], BF, tag="wo_sb1")
        for fc in range(8):
            nc.gpsimd.dma_start(out=wo_sb0[:, fc, :], in_=wo_d[:, fc, 0:512])
        for fc in range(8):
            nc.scalar.dma_start(out=wo_sb1[:, fc, :], in_=wo_d[:, fc, 512:1024])

        # ---- K: project own half, AllGather in two 4-head-pair chunks ----
        # k tiles [128 pair-stacked dk, 4 hp, 2048 kpos]; own half staged in
        # cols 0:1024, then the full gathered tensor (global kpos order)
        # overwrites the whole tile.
        k_lo = kp.tile([P, 4, S], BF, tag="k_lo")
        k_hi = kp.tile([P, 4, S], BF, tag="k_hi")

        def emit_kproj(k_tile, hp_base):
            for hpo in range(4):
                hp = hp_base + hpo
                hsl = ts(hp, P)
                for col in range(2):
                    psk = zp.tile([P, 512], F32, tag="z")
                    for d in range(8):
                        nc.tensor.matmul(psk[:, :], wk_sb[:, d, hsl],
                                         xkT[:, d, ts(col, 512)],
                                         start=(d == 0), stop=(d == 7))
                    nc.vector.tensor_scalar_add(
                        k_tile[:, hpo, ds(col * 512, 512)], psk[:, :],
                        bkc[:, hp:hp + 1])

        def kgather(k_tile, kin, kout):
            nc.sync.dma_start(out=kin[:, :], in_=k_tile[:, :, 0:SQ])
            nc.gpsimd.collective_compute(
                "AllGather", mybir.AluOpType.bypass, replica_groups=PAIRS,
                ins=[kin.opt()], outs=[kout.opt()])
            kv = k_tile.rearrange("p h (s x) -> p h s x", s=2)
            for s_ in range(2):
                nc.sync.dma_start(
                    out=kv[:, :, s_, :],
                    in_=kout[s_].rearrange("p (h x) -> p h x", h=4))

        kin0 = dram.tile([P, 4 * SQ], BF, tag="kin0")
        kout0 = dram.tile([2, P, 4 * SQ], BF, tag="kout0")
        kin1 = dram.tile([P, 4 * SQ], BF, tag="kin1")
        kout1 = dram.tile([2, P, 4 * SQ], BF, tag="kout1")

        # ---- V: project own half per quad into v_send, AllGather ----
        vsp = ctx.enter_context(tc.tile_pool(name="vsp", bufs=1))
        v_send = vsp.tile([P, SQ // P, 4, 130], BF, tag="v_send")
        vqp = ctx.enter_context(tc.tile_pool(name="vqp", bufs=1))
        v_q = vqp.tile([P, S // P, 4, 130], BF, tag="v_q")
        vin0 = dram.tile([P, 8 * 4 * 130], BF, tag="vin0")
        vin1 = dram.tile([P, 8 * 4 * 130], BF, tag="vin1")
        vout0 = dram.tile([2, P, 8 * 4 * 130], BF, tag="vout0")
        vout1 = dram.tile([2, P, 8 * 4 * 130], BF, tag="vout1")
        vin = [vin0, vin1]
        vout = [vout0, vout1]

        def emit_vproj(q4):
            vsv = v_send.rearrange("p s h (a c) -> p s h a c", a=2)
            nc.vector.memset(vsv[:, :, :, :, 64:65], 1.0)
            q4sl = ds(q4 * 512, 512)
            for sc in range(SQ // P):
                psv = zp.tile([P, 512], F32, tag="z")
                for dc in range(8):
                    nc.tensor.matmul(psv[:, :], xvT[:, dc, ts(sc, P)],
                                     wv_sb[:, dc, q4sl],
                                     start=(dc == 0), stop=(dc == 7))
                nc.vector.tensor_add(
                    vsv[:, sc, :, :, 0:64],
                    psv[:, :].rearrange("p (h a c) -> p h a c", h=4, a=2),
                    bvb[:, q4sl].rearrange("p (h a c) -> p h a c", h=4, a=2),
                )
            nc.scalar.dma_start(out=vin[q4][:, :], in_=v_send[:, :, :, :])
            nc.gpsimd.collective_compute(
                "AllGather", mybir.AluOpType.bypass, replica_groups=PAIRS,
                ins=[vin[q4].opt()], outs=[vout[q4].opt()])

        def emit_vrecv(q4):
            for s_ in range(2):
                nc.scalar.dma_start(
                    out=v_q[:, ds(s_ * 8, 8), :, :],
                    in_=vout[q4][s_].rearrange("p (s h c) -> p s h c", s=8, h=4))

        # z^T accumulator: [128 = dv(h0)|dv(h1), 8 head-pairs, 1024 q]
        zT = ztp.tile([P, NHP, SQ], BF, tag="zT")

        def outproj_step(qc, dt, wo_sb, pool=None, ptag="proj"):
            def emit():
                ps = (pool or pp).tile([P, 512], F32, tag=ptag)
                for fc in range(8):
                    nc.tensor.matmul(ps[:, :], zT[:, fc, ts(qc, P)],
                                     wo_sb[:, fc, :],
                                     start=(fc == 0), stop=(fc == 7))
                o_t = opool.tile([P, 512], F32, tag="o")
                nc.vector.tensor_add(o_t[:, :], ps[:, :], bo_bc[:, ts(dt, 512)])
                nc.sync.dma_start(out=out[ts(qc, P), ts(dt, 512)], in_=o_t[:, :])
            return emit

        op_early = [outproj_step(qc, 0, wo_sb0) for qc in range(4)]
        op_early += [outproj_step(qc, 1, wo_sb1) for qc in range(4)]

        def make_qproj_steps(hp, hsl):
            """Closures emitting HALF psum-groups (4 mms) of the q^T
            projection for head-pair hp — woven into the previous
            head-pair's attention."""
            q_t = qkv.tile([P, SQ], BF, tag="q_t")
            steps = []
            state = {}

            def half(col, lo):
                def emit():
                    if lo:
                        ps = pp.tile([P, 512], F32, tag="proj")
                        state[col] = ps
                    else:
                        ps = state.pop(col)
                    for dc in range(4):
                        d = dc if lo else dc + 4
                        nc.tensor.matmul(ps[:, :], wq_sb[:, d, hsl],
                                         xqT[:, d, ds(col, 512)],
                                         start=(d == 0), stop=(d == 7))
                    if not lo:
                        nc.vector.tensor_scalar_add(
                            q_t[:, ds(col, 512)], ps[:, :], bqc[:, hp:hp + 1])
                return emit

            for qt in range(SQ // 512):
                for lo in (True, False):
                    steps.append(half(qt * 512, lo))
            return q_t, steps

        NJ = S // P  # 16 k-chunks
        # ---- phase A ----
        # hp0+hp1 q-projections up-front (start PE on the earliest loads,
        # cover collective flight); then both V projections and the two K
        # chunks with AllGather triggers interleaved V0,K0,V1,K1 — the
        # pieces hp0 needs (v0, k_lo) fly first while K-proj/V-proj cover.
        # Thereafter distance-2 weave: hp+2's q-proj woven into hp's
        # attention (qkv bufs=3 holds in-use/ready/being-built).
        q_ready = {}
        for h0 in range(2):
            q_ready[h0], steps = make_qproj_steps(h0, ts(h0, P))
            for s_ in steps:
                s_()
        emit_vproj(0)
        emit_kproj(k_lo, 0)
        kgather(k_lo, kin0, kout0)
        emit_vproj(1)
        emit_kproj(k_hi, 4)
        kgather(k_hi, kin1, kout1)
        emit_vrecv(0)
        for q4 in range(NHP // 4):
            k_tile = k_lo if q4 == 0 else k_hi
            if q4 > 0:
                emit_vrecv(q4)
            for hp_i in range(4):
                hp = q4 * 4 + hp_i
                q_t = q_ready.pop(hp)

                next_steps = []
                if hp + 2 < NHP:
                    nhp = hp + 2
                    q_ready[nhp], next_steps = make_qproj_steps(nhp, ts(nhp, P))
                pending = list(next_steps)

                # ---- attention for the two heads of this pair ----
                for qt in range(SQ // 512):
                    qsl = ts(qt, 512)
                    ps_z0 = zp.tile([P, 512], F32, tag="z")
                    ps_z1 = zp.tile([P, 512], F32, tag="z")
                    for j2 in range(NJ // 2):
                        ps_s0 = sp.tile([P, 1024], F32, tag="s")
                        ps_s1 = sp.tile([P, 1024], F32, tag="s")
                        for i in range(2):
                            j = 2 * j2 + i
                            nc.tensor.matmul(ps_s0[:, ts(i, 512)],
                                             k_tile[0:64, hp_i, ts(j, P)],
                                             q_t[0:64, qsl],
                                             start=True, stop=True)
                            nc.tensor.matmul(ps_s1[:, ts(i, 512)],
                                             k_tile[64:P, hp_i, ts(j, P)],
                                             q_t[64:P, qsl],
                                             start=True, stop=True)
                        e0 = epool.tile([P, 1024], BF, tag="e")
                        e1 = epool.tile([P, 1024], BF, tag="e")
                        nc.scalar.activation(e0[:, :], ps_s0[:, :], EXP)
                        nc.scalar.activation(e1[:, :], ps_s1[:, :], EXP)
                        if "noweave" not in VARIANT:
                            if j2 % 2 == 1 and pending:
                                pending.pop(0)()
                            elif hp == NHP - 1 and qt == 1 and op_early:
                                op_early.pop(0)()
                        for i in range(2):
                            j = 2 * j2 + i
                            nc.tensor.matmul(ps_z0[0:65, :],
                                             v_q[:, j, hp_i, 0:65],
                                             e0[:, ts(i, 512)],
                                             start=(j == 0), stop=(j == NJ - 1))
                            nc.tensor.matmul(ps_z1[0:65, :],
                                             v_q[:, j, hp_i, 65:130],
                                             e1[:, ts(i, 512)],
                                             start=(j == 0), stop=(j == NJ - 1))
                    # normalize: z = z' * recip(rowsum). approx_fast (~5x
                    # cheaper, 18 bits) faults on PSUM sources -> stage the
                    # rowsum row to SBUF first; short DVE ops also keep the
                    # in-order DVE queue from stalling woven psum drains.
                    r0 = rpool.tile([1, 512], F32, tag="r")
                    nc.vector.tensor_copy(r0[0:1, :], ps_z0[64:65, :])
                    nc.vector.reciprocal_approx_fast(r0[0:1, :], r0[0:1, :])
                    rb0 = rpool.tile([64, 512], F32, tag="rb")
                    nc.gpsimd.partition_broadcast(rb0[:, :], r0[0:1, :])
                    nc.vector.tensor_mul(zT[0:64, hp, qsl], ps_z0[0:64, :], rb0[:, :])
                    r1 = rpool.tile([1, 512], F32, tag="r")
                    nc.vector.tensor_copy(r1[0:1, :], ps_z1[64:65, :])
                    nc.vector.reciprocal_approx_fast(r1[0:1, :], r1[0:1, :])
                    rb1 = rpool.tile([64, 512], F32, tag="rb")
                    nc.gpsimd.partition_broadcast(rb1[:, :], r1[0:1, :])
                    nc.vector.tensor_mul(zT[64:P, hp, qsl], ps_z1[0:64, :], rb1[:, :])
                for s_ in pending:
                    s_()

        # ---- output projection (remainder) ----
        for s_ in op_early:
            s_()
        for dt, wo_sb in ((0, wo_sb0), (1, wo_sb1)):
            for qc in range(4, SQ // P):
                outproj_step(qc, dt, wo_sb, pool=zp, ptag="z")()


_NC_CACHE = {}


def get_nc(variant=None):
    if variant is None:
        variant = os.environ.get("KVARIANT", "")
    if variant not in _NC_CACHE:
        nc = bacc.Bacc("TRN2", target_bir_lowering=False, debug=False,
                       num_devices=N_CORES)
        with tile.TileContext(nc) as tc:
            build_kernel(nc, tc, variant)
        nc.compile()
        _NC_CACHE[variant] = nc
    return _NC_CACHE[variant]


def _bf(a):
    return np.ascontiguousarray(a.astype(ml_dtypes.bfloat16))


def shard_inputs(inputs):
    f = lambda n: np.asarray(inputs[n], dtype=np.float32)
    iq, ik, iv = f("input_query"), f("input_key"), f("input_value")
    wq, wk, wv = f("Wq"), f("Wk"), f("Wv")
    bq, bk, bv = f("bq"), f("bk"), f("bv")
    wo, bo = f("Wo"), f("bo")

    # weights -> [128 d-in-chunk, 8 dchunk, (h k)] bf16
    def pack_w(w):  # [H, D, DK]
        x = np.transpose(w, (1, 0, 2)).reshape(8, P, H * DK)  # [dc, dp, (h k)]
        return _bf(np.transpose(x, (1, 0, 2)))  # [128, 8, H*DK]

    # biases -> [128 pair-stacked, 8 hp] f32
    def pack_b(b):  # [H, DK]
        x = b.reshape(NHP, 2 * DK).T  # [128, NHP]
        return np.ascontiguousarray(x)

    shared = {
        "wq": pack_w(wq), "wk": pack_w(wk), "wv": pack_w(wv),
        "bq": pack_b(bq), "bk": pack_b(bk),
        "bvr": _bf(bv.reshape(1, H * DV)),
        "wo": _bf(np.transpose(wo.reshape(8, P, D), (1, 0, 2))),
        "bo": _bf(bo[None, :]),
    }
    in_maps = []
    for c in range(N_CORES):
        b_, half = c // 2, c % 2
        m = dict(shared)
        m["xqT"] = _bf(iq[b_, half * SQ:(half + 1) * SQ, :].T)
        m["xkT"] = _bf(ik[b_, half * SQ:(half + 1) * SQ, :].T)
        m["xvT"] = _bf(iv[b_, half * SQ:(half + 1) * SQ, :].T)
        in_maps.append(m)
    return in_maps


def kernel(**inputs):
    nc = get_nc()
    in_maps = shard_inputs(inputs)
    res = run_bass_kernel_spmd(nc, in_maps, core_ids=list(range(N_CORES)),
                               trace=False)
    out = np.empty((B, S, D), np.float32)
    for c in range(N_CORES):
        b_, half = c // 2, c % 2
        out[b_, half * SQ:(half + 1) * SQ, :] = res.results[c]["out"]
    return out


# revision 41
# speedup vs baseline: 1.1502x; 1.1502x over previous
"""MHA kernel for Trainium2, 8 NeuronCores.

Sharding: (batch, query-half) -> 8 shards. Core c handles batch c//2,
query rows (c%2)*1024:(c%2+1)*1024, all 16 heads. K/V projections are
SPLIT between the two cores sharing a batch (by kpos / s half) and
exchanged with pair-wise AllGather collectives (replica groups
[[0,1],[2,3],[4,5],[6,7]]), halving the K/V projection matmul work.
AllGather output is rank-ordered identically on both cores, giving both
the full K/V in global kpos order (attention is permutation-invariant
over kpos anyway). Output rows are disjoint across cores.

Host-side marshaling: X slices transposed and cast to bf16 ([D, S*]
layout, K/V halves per core); weights pre-packed into SBUF layouts.

Per-core compute (bf16 matmuls, f32 PSUM):
  K proj (own half, all 8 head-pairs) up-front -> AllGather in 2 chunks;
  V proj (own half) per quad -> AllGather; quad1's V proj emitted before
  head-pair 3's attention so the collective overlaps it.
  Per head-pair: q^T via woven half-psum-groups; scores^T per head via
  K=64 matmuls (head pair concurrent in PE row-tile groups);
  E = exp(scores^T) on ScalarE; z'^T accumulated with the ones-column
  trick; z^T = z'^T[:64] * recip(z'^T[64]) (approx_fast via SBUF staging,
  GpSimd partition-broadcast); out = z^T.T @ Wo + bo.
"""

import os

import numpy as np
import ml_dtypes



import concourse.bass as bass
import concourse.tile as tile
from concourse import bacc, mybir
from concourse.bass import ds, ts
from concourse.bass_utils import run_bass_kernel_spmd

B, S, D = 4, 2048, 1024
H, DK, DV = 16, 64, 64
N_CORES = 8
SQ = S // 2  # query rows per core
P = 128
NHP = H // 2  # head pairs
F32 = mybir.dt.float32
BF = mybir.dt.bfloat16
EXP = mybir.ActivationFunctionType.Exp
PAIRS = [[0, 1], [2, 3], [4, 5], [6, 7]]


def build_kernel(nc, tc, VARIANT=""):
    # pre-transposed bf16 inputs (K/V are the core's HALF: kpos/s slice)
    xqT_d = nc.declare_dram_parameter("xqT", [D, SQ], BF, isOutput=False).ap()
    xkT_d = nc.declare_dram_parameter("xkT", [D, SQ], BF, isOutput=False).ap()
    xvT_d = nc.declare_dram_parameter("xvT", [D, SQ], BF, isOutput=False).ap()
    # weights pre-packed [128, 8 dchunk, 1024 (h k)] bf16
    wq_d = nc.declare_dram_parameter("wq", [P, 8, H * DK], BF, isOutput=False).ap()
    wk_d = nc.declare_dram_parameter("wk", [P, 8, H * DK], BF, isOutput=False).ap()
    wv_d = nc.declare_dram_parameter("wv", [P, 8, H * DV], BF, isOutput=False).ap()
    # biases pre-packed [128 (pair-stacked), 8 hp] f32
    bq_d = nc.declare_dram_parameter("bq", [P, NHP], F32, isOutput=False).ap()
    bk_d = nc.declare_dram_parameter("bk", [P, NHP], F32, isOutput=False).ap()
    bvr_d = nc.declare_dram_parameter("bvr", [1, H * DV], BF, isOutput=False).ap()
    # Wo pre-packed [128, 8 fchunk, 1024 dout] bf16; bo [1, D] bf16
    wo_d = nc.declare_dram_parameter("wo", [P, 8, D], BF, isOutput=False).ap()
    bo_d = nc.declare_dram_parameter("bo", [1, D], BF, isOutput=False).ap()
    out = nc.declare_dram_parameter("out", [SQ, D], F32, isOutput=True).ap()

    import contextlib

    ctx = contextlib.ExitStack()
    with ctx:
        consts = ctx.enter_context(tc.tile_pool(name="consts", bufs=1))
        wpool = ctx.enter_context(tc.tile_pool(name="wpool", bufs=1))
        xtp = ctx.enter_context(tc.tile_pool(name="xtp", bufs=1))
        kp = ctx.enter_context(tc.tile_pool(name="kp", bufs=1))
        ztp = ctx.enter_context(tc.tile_pool(name="ztp", bufs=1))
        qkv = ctx.enter_context(tc.tile_pool(name="qkv", bufs=3))
        epool = ctx.enter_context(tc.tile_pool(name="epool", bufs=2))
        rpool = ctx.enter_context(tc.tile_pool(name="rpool", bufs=1))
        opool = ctx.enter_context(tc.tile_pool(name="opool", bufs=2))
        wopool = ctx.enter_context(tc.tile_pool(name="wopool", bufs=1))
        dram = ctx.enter_context(tc.tile_pool(name="dram", bufs=1, space="DRAM"))
        # PSUM: 8 banks = pp 1 + sp 4 + zp 3. zp=3 lets the next qt's AV
        # accumulation start while the previous qt's normalization chain
        # is still draining.
        pp = ctx.enter_context(tc.tile_pool(name="pp", bufs=1, space=bass.MemorySpace.PSUM))
        sp = ctx.enter_context(tc.tile_pool(name="sp", bufs=2, space=bass.MemorySpace.PSUM))
        zp = ctx.enter_context(tc.tile_pool(name="zp", bufs=3, space=bass.MemorySpace.PSUM))

        # ---- constants ----
        bqc = consts.tile([P, NHP], F32, tag="bqc")
        bkc = consts.tile([P, NHP], F32, tag="bkc")
        nc.gpsimd.dma_start(out=bqc[:, :], in_=bq_d[:, :])
        nc.gpsimd.dma_start(out=bkc[:, :], in_=bk_d[:, :])
        # staging rows ride the rpool "rb" ring slot (first real rb use is
        # far later; bufs=1 ring serializes the overwrites correctly)
        bvr = rpool.tile([1, D], BF, tag="rb")
        nc.gpsimd.dma_start(out=bvr[0:1, 0:H * DV], in_=bvr_d[0:1, :])
        bvb = consts.tile([P, H * DV], BF, tag="bvb")
        nc.gpsimd.partition_broadcast(bvb[:, :], bvr[0:1, 0:H * DV])
        bo_row = rpool.tile([1, D], BF, tag="rb")
        nc.gpsimd.dma_start(out=bo_row[0:1, :], in_=bo_d[0:1, :])
        bo_bc = consts.tile([P, D], BF, tag="bo_bc")
        nc.gpsimd.partition_broadcast(bo_bc[:, :], bo_row[0:1, :])

        wq_sb = wpool.tile([P, 8, H * DK], BF, tag="wq_sb")
        wk_sb = wpool.tile([P, 8, H * DK], BF, tag="wk_sb")
        wv_sb = wpool.tile([P, 8, H * DV], BF, tag="wv_sb")

        xqT = xtp.tile([P, 8, SQ], BF, tag="xqT")
        xkT = xtp.tile([P, 8, SQ], BF, tag="xkT")
        xvT = xtp.tile([P, 8, SQ], BF, tag="xvT")

        # V-proj gates the first (longest-flight) AllGather: wv + xvT first
        nc.gpsimd.dma_start(out=wv_sb[:, :, :], in_=wv_d[:, :, :])
        for dc in range(8):
            eng = nc.sync if dc % 2 == 0 else nc.scalar
            eng.dma_start(out=xvT[:, dc, :], in_=xvT_d[ds(dc * P, P), :])
        nc.gpsimd.dma_start(out=wk_sb[:, :, :], in_=wk_d[:, :, :])
        for dc in range(8):
            eng = nc.sync if dc % 2 == 0 else nc.scalar
            eng.dma_start(out=xkT[:, dc, :], in_=xkT_d[ds(dc * P, P), :])
        nc.gpsimd.dma_start(out=wq_sb[:, 0:4, :], in_=wq_d[:, 0:4, :])
        for dc in range(4):
            eng = nc.sync if dc % 2 == 0 else nc.scalar
            eng.dma_start(out=xqT[:, dc, :], in_=xqT_d[ds(dc * P, P), :])
        nc.gpsimd.dma_start(out=wq_sb[:, 4:8, :], in_=wq_d[:, 4:8, :])
        for dc in range(4, 8):
            eng = nc.sync if dc % 2 == 0 else nc.scalar
            eng.dma_start(out=xqT[:, dc, :], in_=xqT_d[ds(dc * P, P), :])

        wo_sb0 = wopool.tile([P, 8, 512], BF, tag="wo_sb")
        wo_sb1 = wopool.tile([P, 8, 512], BF, tag="wo_sb1")
        for fc in range(8):
            nc.gpsimd.dma_start(out=wo_sb0[:, fc, :], in_=wo_d[:, fc, 0:512])
        for fc in range(8):
            nc.scalar.dma_start(out=wo_sb1[:, fc, :], in_=wo_d[:, fc, 512:1024])

        # ---- K: project own half, AllGather in two 4-head-pair chunks ----
        # k tiles [128 pair-stacked dk, 4 hp, 2048 kpos]; own half staged in
        # cols 0:1024, then the full gathered tensor (global kpos order)
        # overwrites the whole tile.
        k_lo = kp.tile([P, 4, S], BF, tag="k_lo")
        k_hi = kp.tile([P, 4, S], BF, tag="k_hi")

        def emit_kproj(k_tile, hp_base):
            for hpo in range(4):
                hp = hp_base + hpo
                hsl = ts(hp, P)
                for col in range(2):
                    psk = zp.tile([P, 512], F32, tag="z")
                    for d in range(8):
                        nc.tensor.matmul(psk[:, :], wk_sb[:, d, hsl],
                                         xkT[:, d, ts(col, 512)],
                                         start=(d == 0), stop=(d == 7))
                    nc.vector.tensor_scalar_add(
                        k_tile[:, hpo, ds(col * 512, 512)], psk[:, :],
                        bkc[:, hp:hp + 1])

        def kgather(k_tile, kin, kout):
            nc.sync.dma_start(out=kin[:, :], in_=k_tile[:, :, 0:SQ])
            nc.gpsimd.collective_compute(
                "AllGather", mybir.AluOpType.bypass, replica_groups=PAIRS,
                ins=[kin.opt()], outs=[kout.opt()])
            kv = k_tile.rearrange("p h (s x) -> p h s x", s=2)
            for s_ in range(2):
                nc.sync.dma_start(
                    out=kv[:, :, s_, :],
                    in_=kout[s_].rearrange("p (h x) -> p h x", h=4))

        kin0 = dram.tile([P, 4 * SQ], BF, tag="kin0")
        kout0 = dram.tile([2, P, 4 * SQ], BF, tag="kout0")
        kin1 = dram.tile([P, 4 * SQ], BF, tag="kin1")
        kout1 = dram.tile([2, P, 4 * SQ], BF, tag="kout1")

        # ---- V: project own half per quad into v_send, AllGather ----
        vsp = ctx.enter_context(tc.tile_pool(name="vsp", bufs=1))
        v_send = vsp.tile([P, SQ // P, 4, 130], BF, tag="v_send")
        vqp = ctx.enter_context(tc.tile_pool(name="vqp", bufs=1))
        v_q = vqp.tile([P, S // P, 4, 130], BF, tag="v_q")
        vin0 = dram.tile([P, 8 * 4 * 130], BF, tag="vin0")
        vin1 = dram.tile([P, 8 * 4 * 130], BF, tag="vin1")
        vout0 = dram.tile([2, P, 8 * 4 * 130], BF, tag="vout0")
        vout1 = dram.tile([2, P, 8 * 4 * 130], BF, tag="vout1")
        vin = [vin0, vin1]
        vout = [vout0, vout1]

        def emit_vproj(q4):
            vsv = v_send.rearrange("p s h (a c) -> p s h a c", a=2)
            nc.vector.memset(vsv[:, :, :, :, 64:65], 1.0)
            q4sl = ds(q4 * 512, 512)
            for sc in range(SQ // P):
                psv = zp.tile([P, 512], F32, tag="z")
                for dc in range(8):
                    nc.tensor.matmul(psv[:, :], xvT[:, dc, ts(sc, P)],
                                     wv_sb[:, dc, q4sl],
                                     start=(dc == 0), stop=(dc == 7))
                nc.vector.tensor_add(
                    vsv[:, sc, :, :, 0:64],
                    psv[:, :].rearrange("p (h a c) -> p h a c", h=4, a=2),
                    bvb[:, q4sl].rearrange("p (h a c) -> p h a c", h=4, a=2),
                )
            nc.scalar.dma_start(out=vin[q4][:, :], in_=v_send[:, :, :, :])
            nc.gpsimd.collective_compute(
                "AllGather", mybir.AluOpType.bypass, replica_groups=PAIRS,
                ins=[vin[q4].opt()], outs=[vout[q4].opt()])

        def emit_vrecv(q4):
            for s_ in range(2):
                nc.scalar.dma_start(
                    out=v_q[:, ds(s_ * 8, 8), :, :],
                    in_=vout[q4][s_].rearrange("p (s h c) -> p s h c", s=8, h=4))

        # z^T accumulator: [128 = dv(h0)|dv(h1), 8 head-pairs, 1024 q]
        zT = ztp.tile([P, NHP, SQ], BF, tag="zT")

        def outproj_step(qc, dt, wo_sb, pool=None, ptag="proj"):
            def emit():
                ps = (pool or pp).tile([P, 512], F32, tag=ptag)
                for fc in range(8):
                    nc.tensor.matmul(ps[:, :], zT[:, fc, ts(qc, P)],
                                     wo_sb[:, fc, :],
                                     start=(fc == 0), stop=(fc == 7))
                o_t = opool.tile([P, 512], F32, tag="o")
                nc.vector.tensor_add(o_t[:, :], ps[:, :], bo_bc[:, ts(dt, 512)])
                nc.sync.dma_start(out=out[ts(qc, P), ts(dt, 512)], in_=o_t[:, :])
            return emit

        op_early = [outproj_step(qc, 0, wo_sb0) for qc in range(4)]
        op_early += [outproj_step(qc, 1, wo_sb1) for qc in range(4)]

        def make_qproj_steps(hp, hsl):
            """Closures emitting HALF psum-groups (4 mms) of the q^T
            projection for head-pair hp — woven into the previous
            head-pair's attention."""
            q_t = qkv.tile([P, SQ], BF, tag="q_t")
            steps = []
            state = {}

            def half(col, lo):
                def emit():
                    if lo:
                        ps = pp.tile([P, 512], F32, tag="proj")
                        state[col] = ps
                    else:
                        ps = state.pop(col)
                    for dc in range(4):
                        d = dc if lo else dc + 4
                        nc.tensor.matmul(ps[:, :], wq_sb[:, d, hsl],
                                         xqT[:, d, ds(col, 512)],
                                         start=(d == 0), stop=(d == 7))
                    if not lo:
                        nc.vector.tensor_scalar_add(
                            q_t[:, ds(col, 512)], ps[:, :], bqc[:, hp:hp + 1])
                return emit

            for qt in range(SQ // 512):
                for lo in (True, False):
                    steps.append(half(qt * 512, lo))
            return q_t, steps

        NJ = S // P  # 16 k-chunks
        # ---- phase A ----
        # hp0+hp1 q-projections up-front (start PE on the earliest loads,
        # cover collective flight); then both V projections and the two K
        # chunks with AllGather triggers interleaved V0,K0,V1,K1 — the
        # pieces hp0 needs (v0, k_lo) fly first while K-proj/V-proj cover.
        # Thereafter distance-2 weave: hp+2's q-proj woven into hp's
        # attention (qkv bufs=3 holds in-use/ready/being-built).
        emit_vproj(0)
        emit_kproj(k_lo, 0)
        kgather(k_lo, kin0, kout0)
        emit_kproj(k_hi, 4)
        kgather(k_hi, kin1, kout1)
        emit_vrecv(0)
        q_ready = {}
        for h0 in range(2):
            q_ready[h0], steps = make_qproj_steps(h0, ts(h0, P))
            for s_ in steps:
                s_()
        for q4 in range(NHP // 4):
            k_tile = k_lo if q4 == 0 else k_hi
            if q4 > 0:
                emit_vrecv(q4)
            for hp_i in range(4):
                hp = q4 * 4 + hp_i
                q_t = q_ready.pop(hp)

                # quad1's V proj emitted before hp3's attention so the
                # collective overlaps it
                if hp == 3:
                    emit_vproj(1)

                next_steps = []
                if hp + 2 < NHP:
                    nhp = hp + 2
                    q_ready[nhp], next_steps = make_qproj_steps(nhp, ts(nhp, P))
                pending = list(next_steps)

                # ---- attention for the two heads of this pair ----
                for qt in range(SQ // 512):
                    qsl = ts(qt, 512)
                    ps_z0 = zp.tile([P, 512], F32, tag="z")
                    ps_z1 = zp.tile([P, 512], F32, tag="z")
                    for j2 in range(NJ // 2):
                        ps_s0 = sp.tile([P, 1024], F32, tag="s")
                        ps_s1 = sp.tile([P, 1024], F32, tag="s")
                        for i in range(2):
                            j = 2 * j2 + i
                            nc.tensor.matmul(ps_s0[:, ts(i, 512)],
                                             k_tile[0:64, hp_i, ts(j, P)],
                                             q_t[0:64, qsl],
                                             start=True, stop=True)
                            nc.tensor.matmul(ps_s1[:, ts(i, 512)],
                                             k_tile[64:P, hp_i, ts(j, P)],
                                             q_t[64:P, qsl],
                                             start=True, stop=True)
                        e0 = epool.tile([P, 1024], BF, tag="e")
                        e1 = epool.tile([P, 1024], BF, tag="e")
                        nc.scalar.activation(e0[:, :], ps_s0[:, :], EXP)
                        nc.scalar.activation(e1[:, :], ps_s1[:, :], EXP)
                        if "noweave" not in VARIANT:
                            if j2 % 2 == 1 and pending:
                                pending.pop(0)()
                            elif hp == NHP - 1 and qt == 1 and op_early:
                                op_early.pop(0)()
                        for i in range(2):
                            j = 2 * j2 + i
                            nc.tensor.matmul(ps_z0[0:65, :],
                                             v_q[:, j, hp_i, 0:65],
                                             e0[:, ts(i, 512)],
                                             start=(j == 0), stop=(j == NJ - 1))
                            nc.tensor.matmul(ps_z1[0:65, :],
                                             v_q[:, j, hp_i, 65:130],
                                             e1[:, ts(i, 512)],
                                             start=(j == 0), stop=(j == NJ - 1))
                    # normalize: z = z' * recip(rowsum). approx_fast (~5x
                    # cheaper, 18 bits) faults on PSUM sources -> stage the
                    # rowsum row to SBUF first; short DVE ops also keep the
                    # in-order DVE queue from stalling woven psum drains.
                    r0 = rpool.tile([1, 512], F32, tag="r")
                    nc.vector.tensor_copy(r0[0:1, :], ps_z0[64:65, :])
                    nc.vector.reciprocal_approx_fast(r0[0:1, :], r0[0:1, :])
                    rb0 = rpool.tile([64, 512], F32, tag="rb")
                    nc.gpsimd.partition_broadcast(rb0[:, :], r0[0:1, :])
                    nc.vector.tensor_mul(zT[0:64, hp, qsl], ps_z0[0:64, :], rb0[:, :])
                    r1 = rpool.tile([1, 512], F32, tag="r")
                    nc.vector.tensor_copy(r1[0:1, :], ps_z1[64:65, :])
                    nc.vector.reciprocal_approx_fast(r1[0:1, :], r1[0:1, :])
                    rb1 = rpool.tile([64, 512], F32, tag="rb")
                    nc.gpsimd.partition_broadcast(rb1[:, :], r1[0:1, :])
                    nc.vector.tensor_mul(zT[64:P, hp, qsl], ps_z1[0:64, :], rb1[:, :])
                for s_ in pending:
                    s_()

        # ---- output projection (remainder) ----
        for s_ in op_early:
            s_()
        for dt, wo_sb in ((0, wo_sb0), (1, wo_sb1)):
            for qc in range(4, SQ // P):
                outproj_step(qc, dt, wo_sb, pool=zp, ptag="z")()


_NC_CACHE = {}


def get_nc(variant=None):
    if variant is None:
        variant = os.environ.get("KVARIANT", "")
    if variant not in _NC_CACHE:
        nc = bacc.Bacc("TRN2", target_bir_lowering=False, debug=False,
                       num_devices=N_CORES)
        with tile.TileContext(nc) as tc:
            build_kernel(nc, tc, variant)
        nc.compile()
        _NC_CACHE[variant] = nc
    return _NC_CACHE[variant]


def _bf(a):
    return np.ascontiguousarray(a.astype(ml_dtypes.bfloat16))


def shard_inputs(inputs):
    f = lambda n: np.asarray(inputs[n], dtype=np.float32)
    iq, ik, iv = f("input_query"), f("input_key"), f("input_value")
    wq, wk, wv = f("Wq"), f("Wk"), f("Wv")
    bq, bk, bv = f("bq"), f("bk"), f("bv")
    wo, bo = f("Wo"), f("bo")

    # weights -> [128 d-in-chunk, 8 dchunk, (h k)] bf16
    def pack_w(w):  # [H, D, DK]
        x = np.transpose(w, (1, 0, 2)).reshape(8, P, H * DK)  # [dc, dp, (h k)]
        return _bf(np.transpose(x, (1, 0, 2)))  # [128, 8, H*DK]

    # biases -> [128 pair-stacked, 8 hp] f32
    def pack_b(b):  # [H, DK]
        x = b.reshape(NHP, 2 * DK).T  # [128, NHP]
        return np.ascontiguousarray(x)

    shared = {
        "wq": pack_w(wq), "wk": pack_w(wk), "wv": pack_w(wv),
        "bq": pack_b(bq), "bk": pack_b(bk),
        "bvr": _bf(bv.reshape(1, H * DV)),
        "wo": _bf(np.transpose(wo.reshape(8, P, D), (1, 0, 2))),
        "bo": _bf(bo[None, :]),
    }
    in_maps = []
    for c in range(N_CORES):
        b_, half = c // 2, c % 2
        m = dict(shared)
        m["xqT"] = _bf(iq[b_, half * SQ:(half + 1) * SQ, :].T)
        m["xkT"] = _bf(ik[b_, half * SQ:(half + 1) * SQ, :].T)
        m["xvT"] = _bf(iv[b_, half * SQ:(half + 1) * SQ, :].T)
        in_maps.append(m)
    return in_maps


def kernel(**inputs):
    nc = get_nc()
    in_maps = shard_inputs(inputs)
    res = run_bass_kernel_spmd(nc, in_maps, core_ids=list(range(N_CORES)),
                               trace=False)
    out = np.empty((B, S, D), np.float32)
    for c in range(N_CORES):
        b_, half = c // 2, c % 2
        out[b_, half * SQ:(half + 1) * SQ, :] = res.results[c]["out"]
    return out


# revision 46
# speedup vs baseline: 1.1521x; 1.0017x over previous
"""MHA kernel for Trainium2, 8 NeuronCores.

Sharding: (batch, query-half) -> 8 shards. Core c handles batch c//2,
query rows (c%2)*1024:(c%2+1)*1024, all 16 heads. K/V projections are
SPLIT between the two cores sharing a batch (by kpos / s half) and
exchanged with pair-wise AllGather collectives (replica groups
[[0,1],[2,3],[4,5],[6,7]]), halving the K/V projection matmul work.
AllGather output is rank-ordered identically on both cores, giving both
the full K/V in global kpos order (attention is permutation-invariant
over kpos anyway). Output rows are disjoint across cores.

Host-side marshaling: X slices transposed and cast to bf16 ([D, S*]
layout, K/V halves per core); weights pre-packed into SBUF layouts.

Per-core compute (bf16 matmuls, f32 PSUM):
  K proj (own half, all 8 head-pairs) up-front -> AllGather in 2 chunks;
  V proj (own half) per quad -> AllGather; quad1's V proj emitted before
  head-pair 3's attention so the collective overlaps it.
  Per head-pair: q^T via woven half-psum-groups; scores^T per head via
  K=64 matmuls (head pair concurrent in PE row-tile groups);
  E = exp(scores^T) on ScalarE; z'^T accumulated with the ones-column
  trick; z^T = z'^T[:64] * recip(z'^T[64]) (approx_fast via SBUF staging,
  GpSimd partition-broadcast); out = z^T.T @ Wo + bo.
"""

import os

import numpy as np
import ml_dtypes



import concourse.bass as bass
import concourse.tile as tile
from concourse import bacc, mybir
from concourse.bass import ds, ts
from concourse.bass_utils import run_bass_kernel_spmd

B, S, D = 4, 2048, 1024
H, DK, DV = 16, 64, 64
N_CORES = 8
SQ = S // 2  # query rows per core
P = 128
NHP = H // 2  # head pairs
F32 = mybir.dt.float32
BF = mybir.dt.bfloat16
EXP = mybir.ActivationFunctionType.Exp
PAIRS = [[0, 1], [2, 3], [4, 5], [6, 7]]


def build_kernel(nc, tc, VARIANT=""):
    # pre-transposed bf16 inputs (K/V are the core's HALF: kpos/s slice)
    xqT_d = nc.declare_dram_parameter("xqT", [D, SQ], BF, isOutput=False).ap()
    xkT_d = nc.declare_dram_parameter("xkT", [D, SQ], BF, isOutput=False).ap()
    xvT_d = nc.declare_dram_parameter("xvT", [D, SQ], BF, isOutput=False).ap()
    # weights pre-packed [128, 8 dchunk, 1024 (h k)] bf16
    wq_d = nc.declare_dram_parameter("wq", [P, 8, H * DK], BF, isOutput=False).ap()
    wk_d = nc.declare_dram_parameter("wk", [P, 8, H * DK], BF, isOutput=False).ap()
    wv_d = nc.declare_dram_parameter("wv", [P, 8, H * DV], BF, isOutput=False).ap()
    # biases pre-packed [128 (pair-stacked), 8 hp] f32
    bq_d = nc.declare_dram_parameter("bq", [P, NHP], F32, isOutput=False).ap()
    bk_d = nc.declare_dram_parameter("bk", [P, NHP], F32, isOutput=False).ap()
    bvr_d = nc.declare_dram_parameter("bvr", [1, H * DV], BF, isOutput=False).ap()
    # Wo pre-packed [128, 8 fchunk, 1024 dout] bf16; bo [1, D] bf16
    wo_d = nc.declare_dram_parameter("wo", [P, 8, D], BF, isOutput=False).ap()
    bo_d = nc.declare_dram_parameter("bo", [1, D], BF, isOutput=False).ap()
    out = nc.declare_dram_parameter("out", [SQ, D], F32, isOutput=True).ap()

    import contextlib

    ctx = contextlib.ExitStack()
    with ctx:
        consts = ctx.enter_context(tc.tile_pool(name="consts", bufs=1))
        wpool = ctx.enter_context(tc.tile_pool(name="wpool", bufs=1))
        xtp = ctx.enter_context(tc.tile_pool(name="xtp", bufs=1))
        kp = ctx.enter_context(tc.tile_pool(name="kp", bufs=1))
        ztp = ctx.enter_context(tc.tile_pool(name="ztp", bufs=1))
        qkv = ctx.enter_context(tc.tile_pool(name="qkv", bufs=3))
        epool = ctx.enter_context(tc.tile_pool(name="epool", bufs=2))
        rpool = ctx.enter_context(tc.tile_pool(name="rpool", bufs=1))
        opool = ctx.enter_context(tc.tile_pool(name="opool", bufs=2))
        wopool = ctx.enter_context(tc.tile_pool(name="wopool", bufs=1))
        dram = ctx.enter_context(tc.tile_pool(name="dram", bufs=1, space="DRAM"))
        # PSUM: 8 banks = pp 1 + sp 4 + zp 3. zp=3 lets the next qt's AV
        # accumulation start while the previous qt's normalization chain
        # is still draining.
        pp = ctx.enter_context(tc.tile_pool(name="pp", bufs=1, space=bass.MemorySpace.PSUM))
        sp = ctx.enter_context(tc.tile_pool(name="sp", bufs=2, space=bass.MemorySpace.PSUM))
        zp = ctx.enter_context(tc.tile_pool(name="zp", bufs=3, space=bass.MemorySpace.PSUM))

        # ---- constants ----
        bqc = consts.tile([P, NHP], F32, tag="bqc")
        bkc = consts.tile([P, NHP], F32, tag="bkc")
        nc.gpsimd.dma_start(out=bqc[:, :], in_=bq_d[:, :])
        nc.gpsimd.dma_start(out=bkc[:, :], in_=bk_d[:, :])
        # staging rows ride the rpool "rb" ring slot (first real rb use is
        # far later; bufs=1 ring serializes the overwrites correctly)
        bvr = rpool.tile([1, D], BF, tag="rb")
        nc.gpsimd.dma_start(out=bvr[0:1, 0:H * DV], in_=bvr_d[0:1, :])
        bvb = consts.tile([P, H * DV], BF, tag="bvb")
        nc.gpsimd.partition_broadcast(bvb[:, :], bvr[0:1, 0:H * DV])
        bo_row = rpool.tile([1, D], BF, tag="rb")
        nc.gpsimd.dma_start(out=bo_row[0:1, :], in_=bo_d[0:1, :])
        bo_bc = consts.tile([P, D], BF, tag="bo_bc")
        nc.gpsimd.partition_broadcast(bo_bc[:, :], bo_row[0:1, :])

        wq_sb = wpool.tile([P, 8, H * DK], BF, tag="wq_sb")
        wk_sb = wpool.tile([P, 8, H * DK], BF, tag="wk_sb")
        wv_sb = wpool.tile([P, 8, H * DV], BF, tag="wv_sb")

        xqT = xtp.tile([P, 8, SQ], BF, tag="xqT")
        xkT = xtp.tile([P, 8, SQ], BF, tag="xkT")
        xvT = xtp.tile([P, 8, SQ], BF, tag="xvT")

        # V-proj gates the first (longest-flight) AllGather: wv + xvT first,
        # spread across 4 queues (vector issues DMAs too) to shorten the ramp
        nc.gpsimd.dma_start(out=wv_sb[:, 0:4, :], in_=wv_d[:, 0:4, :])
        nc.gpsimd.dma_start(out=wv_sb[:, 4:8, :], in_=wv_d[:, 4:8, :])
        for dc in range(8):
            eng = nc.sync if dc % 2 == 0 else nc.scalar
            eng.dma_start(out=xvT[:, dc, :], in_=xvT_d[ds(dc * P, P), :])
        nc.gpsimd.dma_start(out=wk_sb[:, 0:4, :], in_=wk_d[:, 0:4, :])
        nc.gpsimd.dma_start(out=wk_sb[:, 4:8, :], in_=wk_d[:, 4:8, :])
        for dc in range(8):
            eng = nc.sync if dc % 2 == 0 else nc.scalar
            eng.dma_start(out=xkT[:, dc, :], in_=xkT_d[ds(dc * P, P), :])
        nc.gpsimd.dma_start(out=wq_sb[:, 0:4, :], in_=wq_d[:, 0:4, :])
        for dc in range(4):
            eng = nc.sync if dc % 2 == 0 else nc.scalar
            eng.dma_start(out=xqT[:, dc, :], in_=xqT_d[ds(dc * P, P), :])
        nc.gpsimd.dma_start(out=wq_sb[:, 4:8, :], in_=wq_d[:, 4:8, :])
        for dc in range(4, 8):
            eng = nc.sync if dc % 2 == 0 else nc.scalar
            eng.dma_start(out=xqT[:, dc, :], in_=xqT_d[ds(dc * P, P), :])

        wo_sb0 = wopool.tile([P, 8, 512], BF, tag="wo_sb")
        wo_sb1 = wopool.tile([P, 8, 512], BF, tag="wo_sb1")
        for fc in range(8):
            nc.gpsimd.dma_start(out=wo_sb0[:, fc, :], in_=wo_d[:, fc, 0:512])
        for fc in range(8):
            nc.scalar.dma_start(out=wo_sb1[:, fc, :], in_=wo_d[:, fc, 512:1024])

        # ---- K: project own half, AllGather in two 4-head-pair chunks ----
        # k tiles [128 pair-stacked dk, 4 hp, 2048 kpos]; own half staged in
        # cols 0:1024, then the full gathered tensor (global kpos order)
        # overwrites the whole tile.
        k_lo = kp.tile([P, 4, S], BF, tag="k_lo")
        k_hi = kp.tile([P, 4, S], BF, tag="k_hi")

        def emit_kproj(k_tile, hp_base):
            for hpo in range(4):
                hp = hp_base + hpo
                hsl = ts(hp, P)
                for col in range(2):
                    psk = zp.tile([P, 512], F32, tag="z")
                    for d in range(8):
                        nc.tensor.matmul(psk[:, :], wk_sb[:, d, hsl],
                                         xkT[:, d, ts(col, 512)],
                                         start=(d == 0), stop=(d == 7))
                    nc.vector.tensor_scalar_add(
                        k_tile[:, hpo, ds(col * 512, 512)], psk[:, :],
                        bkc[:, hp:hp + 1])

        def kgather(k_tile, kin, kout):
            nc.sync.dma_start(out=kin[:, :], in_=k_tile[:, :, 0:SQ])
            nc.gpsimd.collective_compute(
                "AllGather", mybir.AluOpType.bypass, replica_groups=PAIRS,
                ins=[kin.opt()], outs=[kout.opt()])
            kv = k_tile.rearrange("p h (s x) -> p h s x", s=2)
            # recv slots on separate queues so the two 1MB reads overlap
            for s_, eng in ((0, nc.sync), (1, nc.gpsimd)):
                eng.dma_start(
                    out=kv[:, :, s_, :],
                    in_=kout[s_].rearrange("p (h x) -> p h x", h=4))

        kin0 = dram.tile([P, 4 * SQ], BF, tag="kin0")
        kout0 = dram.tile([2, P, 4 * SQ], BF, tag="kout0")
        kin1 = dram.tile([P, 4 * SQ], BF, tag="kin1")
        kout1 = dram.tile([2, P, 4 * SQ], BF, tag="kout1")

        # ---- V: project own half per quad into v_send, AllGather ----
        vsp = ctx.enter_context(tc.tile_pool(name="vsp", bufs=1))
        v_send = vsp.tile([P, SQ // P, 4, 130], BF, tag="v_send")
        vqp = ctx.enter_context(tc.tile_pool(name="vqp", bufs=1))
        v_q = vqp.tile([P, S // P, 4, 130], BF, tag="v_q")
        vin0 = dram.tile([P, 8 * 4 * 130], BF, tag="vin0")
        vin1 = dram.tile([P, 8 * 4 * 130], BF, tag="vin1")
        vout0 = dram.tile([2, P, 8 * 4 * 130], BF, tag="vout0")
        vout1 = dram.tile([2, P, 8 * 4 * 130], BF, tag="vout1")
        vin = [vin0, vin1]
        vout = [vout0, vout1]

        def emit_vproj(q4):
            vsv = v_send.rearrange("p s h (a c) -> p s h a c", a=2)
            nc.vector.memset(vsv[:, :, :, :, 64:65], 1.0)
            q4sl = ds(q4 * 512, 512)
            for sc in range(SQ // P):
                psv = zp.tile([P, 512], F32, tag="z")
                for dc in range(8):
                    nc.tensor.matmul(psv[:, :], xvT[:, dc, ts(sc, P)],
                                     wv_sb[:, dc, q4sl],
                                     start=(dc == 0), stop=(dc == 7))
                nc.vector.tensor_add(
                    vsv[:, sc, :, :, 0:64],
                    psv[:, :].rearrange("p (h a c) -> p h a c", h=4, a=2),
                    bvb[:, q4sl].rearrange("p (h a c) -> p h a c", h=4, a=2),
                )
            nc.scalar.dma_start(out=vin[q4][:, :], in_=v_send[:, :, :, :])
            nc.gpsimd.collective_compute(
                "AllGather", mybir.AluOpType.bypass, replica_groups=PAIRS,
                ins=[vin[q4].opt()], outs=[vout[q4].opt()])

        def emit_vrecv(q4):
            for s_, eng in ((0, nc.scalar), (1, nc.gpsimd)):
                eng.dma_start(
                    out=v_q[:, ds(s_ * 8, 8), :, :],
                    in_=vout[q4][s_].rearrange("p (s h c) -> p s h c", s=8, h=4))

        # z^T accumulator: [128 = dv(h0)|dv(h1), 8 head-pairs, 1024 q]
        zT = ztp.tile([P, NHP, SQ], BF, tag="zT")

        def outproj_step(qc, dt, wo_sb, pool=None, ptag="proj"):
            def emit():
                ps = (pool or pp).tile([P, 512], F32, tag=ptag)
                for fc in range(8):
                    nc.tensor.matmul(ps[:, :], zT[:, fc, ts(qc, P)],
                                     wo_sb[:, fc, :],
                                     start=(fc == 0), stop=(fc == 7))
                o_t = opool.tile([P, 512], F32, tag="o")
                nc.vector.tensor_add(o_t[:, :], ps[:, :], bo_bc[:, ts(dt, 512)])
                nc.sync.dma_start(out=out[ts(qc, P), ts(dt, 512)], in_=o_t[:, :])
            return emit

        op_early = [outproj_step(qc, 0, wo_sb0) for qc in range(4)]
        op_early += [outproj_step(qc, 1, wo_sb1) for qc in range(4)]

        def make_qproj_steps(hp, hsl):
            """Closures emitting HALF psum-groups (4 mms) of the q^T
            projection for head-pair hp — woven into the previous
            head-pair's attention."""
            q_t = qkv.tile([P, SQ], BF, tag="q_t")
            steps = []
            state = {}

            def half(col, lo):
                def emit():
                    if lo:
                        ps = pp.tile([P, 512], F32, tag="proj")
                        state[col] = ps
                    else:
                        ps = state.pop(col)
                    for dc in range(4):
                        d = dc if lo else dc + 4
                        nc.tensor.matmul(ps[:, :], wq_sb[:, d, hsl],
                                         xqT[:, d, ds(col, 512)],
                                         start=(d == 0), stop=(d == 7))
                    if not lo:
                        nc.vector.tensor_scalar_add(
                            q_t[:, ds(col, 512)], ps[:, :], bqc[:, hp:hp + 1])
                return emit

            for qt in range(SQ // 512):
                for lo in (True, False):
                    steps.append(half(qt * 512, lo))
            return q_t, steps

        NJ = S // P  # 16 k-chunks
        # ---- phase A ----
        # hp0+hp1 q-projections up-front (start PE on the earliest loads,
        # cover collective flight); then both V projections and the two K
        # chunks with AllGather triggers interleaved V0,K0,V1,K1 — the
        # pieces hp0 needs (v0, k_lo) fly first while K-proj/V-proj cover.
        # Thereafter distance-2 weave: hp+2's q-proj woven into hp's
        # attention (qkv bufs=3 holds in-use/ready/being-built).
        emit_vproj(0)
        emit_kproj(k_lo, 0)
        kgather(k_lo, kin0, kout0)
        emit_vrecv(0)
        emit_kproj(k_hi, 4)
        kgather(k_hi, kin1, kout1)
        q_ready = {}
        for h0 in range(2):
            q_ready[h0], steps = make_qproj_steps(h0, ts(h0, P))
            for s_ in steps:
                s_()
        for q4 in range(NHP // 4):
            k_tile = k_lo if q4 == 0 else k_hi
            if q4 > 0:
                emit_vrecv(q4)
            for hp_i in range(4):
                hp = q4 * 4 + hp_i
                q_t = q_ready.pop(hp)

                # quad1's V proj emitted before hp3's attention so the
                # collective overlaps it
                if hp == 3:
                    emit_vproj(1)

                next_steps = []
                if hp + 2 < NHP:
                    nhp = hp + 2
                    q_ready[nhp], next_steps = make_qproj_steps(nhp, ts(nhp, P))
                pending = list(next_steps)

                # ---- attention for the two heads of this pair ----
                for qt in range(SQ // 512):
                    qsl = ts(qt, 512)
                    ps_z0 = zp.tile([P, 512], F32, tag="z")
                    ps_z1 = zp.tile([P, 512], F32, tag="z")
                    for j2 in range(NJ // 2):
                        ps_s0 = sp.tile([P, 1024], F32, tag="s")
                        ps_s1 = sp.tile([P, 1024], F32, tag="s")
                        for i in range(2):
                            j = 2 * j2 + i
                            nc.tensor.matmul(ps_s0[:, ts(i, 512)],
                                             k_tile[0:64, hp_i, ts(j, P)],
                                             q_t[0:64, qsl],
                                             start=True, stop=True)
                            nc.tensor.matmul(ps_s1[:, ts(i, 512)],
                                             k_tile[64:P, hp_i, ts(j, P)],
                                             q_t[64:P, qsl],
                                             start=True, stop=True)
                        e0 = epool.tile([P, 1024], BF, tag="e")
                        e1 = epool.tile([P, 1024], BF, tag="e")
                        nc.scalar.activation(e0[:, :], ps_s0[:, :], EXP)
                        nc.scalar.activation(e1[:, :], ps_s1[:, :], EXP)
                        if "noweave" not in VARIANT:
                            if j2 % 2 == 1 and pending:
                                pending.pop(0)()
                            elif hp == NHP - 1 and qt == 1 and op_early:
                                op_early.pop(0)()
                        for i in range(2):
                            j = 2 * j2 + i
                            nc.tensor.matmul(ps_z0[0:65, :],
                                             v_q[:, j, hp_i, 0:65],
                                             e0[:, ts(i, 512)],
                                             start=(j == 0), stop=(j == NJ - 1))
                            nc.tensor.matmul(ps_z1[0:65, :],
                                             v_q[:, j, hp_i, 65:130],
                                             e1[:, ts(i, 512)],
                                             start=(j == 0), stop=(j == NJ - 1))
                    # normalize: z = z' * recip(rowsum). approx_fast (~5x
                    # cheaper, 18 bits) faults on PSUM sources -> stage the
                    # rowsum row to SBUF first; short DVE ops also keep the
                    # in-order DVE queue from stalling woven psum drains.
                    r0 = rpool.tile([1, 512], F32, tag="r")
                    nc.vector.tensor_copy(r0[0:1, :], ps_z0[64:65, :])
                    nc.vector.reciprocal_approx_fast(r0[0:1, :], r0[0:1, :])
                    rb0 = rpool.tile([64, 512], F32, tag="rb")
                    nc.gpsimd.partition_broadcast(rb0[:, :], r0[0:1, :])
                    nc.vector.tensor_mul(zT[0:64, hp, qsl], ps_z0[0:64, :], rb0[:, :])
                    r1 = rpool.tile([1, 512], F32, tag="r")
                    nc.vector.tensor_copy(r1[0:1, :], ps_z1[64:65, :])
                    nc.vector.reciprocal_approx_fast(r1[0:1, :], r1[0:1, :])
                    rb1 = rpool.tile([64, 512], F32, tag="rb")
                    nc.gpsimd.partition_broadcast(rb1[:, :], r1[0:1, :])
                    nc.vector.tensor_mul(zT[64:P, hp, qsl], ps_z1[0:64, :], rb1[:, :])
                for s_ in pending:
                    s_()

        # ---- output projection (remainder) ----
        for s_ in op_early:
            s_()
        for dt, wo_sb in ((0, wo_sb0), (1, wo_sb1)):
            for qc in range(4, SQ // P):
                outproj_step(qc, dt, wo_sb, pool=zp, ptag="z")()


_NC_CACHE = {}


def get_nc(variant=None):
    if variant is None:
        variant = os.environ.get("KVARIANT", "")
    if variant not in _NC_CACHE:
        nc = bacc.Bacc("TRN2", target_bir_lowering=False, debug=False,
                       num_devices=N_CORES)
        with tile.TileContext(nc) as tc:
            build_kernel(nc, tc, variant)
        nc.compile()
        _NC_CACHE[variant] = nc
    return _NC_CACHE[variant]


def _bf(a):
    return np.ascontiguousarray(a.astype(ml_dtypes.bfloat16))


def shard_inputs(inputs):
    f = lambda n: np.asarray(inputs[n], dtype=np.float32)
    iq, ik, iv = f("input_query"), f("input_key"), f("input_value")
    wq, wk, wv = f("Wq"), f("Wk"), f("Wv")
    bq, bk, bv = f("bq"), f("bk"), f("bv")
    wo, bo = f("Wo"), f("bo")

    # weights -> [128 d-in-chunk, 8 dchunk, (h k)] bf16
    def pack_w(w):  # [H, D, DK]
        x = np.transpose(w, (1, 0, 2)).reshape(8, P, H * DK)  # [dc, dp, (h k)]
        return _bf(np.transpose(x, (1, 0, 2)))  # [128, 8, H*DK]

    # biases -> [128 pair-stacked, 8 hp] f32
    def pack_b(b):  # [H, DK]
        x = b.reshape(NHP, 2 * DK).T  # [128, NHP]
        return np.ascontiguousarray(x)

    shared = {
        "wq": pack_w(wq), "wk": pack_w(wk), "wv": pack_w(wv),
        "bq": pack_b(bq), "bk": pack_b(bk),
        "bvr": _bf(bv.reshape(1, H * DV)),
        "wo": _bf(np.transpose(wo.reshape(8, P, D), (1, 0, 2))),
        "bo": _bf(bo[None, :]),
    }
    in_maps = []
    for c in range(N_CORES):
        b_, half = c // 2, c % 2
        m = dict(shared)
        m["xqT"] = _bf(iq[b_, half * SQ:(half + 1) * SQ, :].T)
        m["xkT"] = _bf(ik[b_, half * SQ:(half + 1) * SQ, :].T)
        m["xvT"] = _bf(iv[b_, half * SQ:(half + 1) * SQ, :].T)
        in_maps.append(m)
    return in_maps


def kernel(**inputs):
    nc = get_nc()
    in_maps = shard_inputs(inputs)
    res = run_bass_kernel_spmd(nc, in_maps, core_ids=list(range(N_CORES)),
                               trace=False)
    out = np.empty((B, S, D), np.float32)
    for c in range(N_CORES):
        b_, half = c // 2, c % 2
        out[b_, half * SQ:(half + 1) * SQ, :] = res.results[c]["out"]
    return out
